# revision 10
# baseline (speedup 1.0000x reference)
"""Trainium2 Bass kernel for nn_Attention_6932077216322.

Multi-head cross-attention + concat-projection + residual + LayerNorm,
returning (out, attns) like the reference.

Sharding: pure data-parallel over (batch, query-row-block): 8 cores,
core c handles batch c//2, query rows (c%2)*1024 .. +1024, all 4 heads,
all 2048 keys. Zero collectives; k/v projections are duplicated between
the two cores of a batch (22% extra PE, beats 2-rank collective cost).

Layout strategy (per core):
  - host pre-transposes memory/decoder_input to (H, S) so projections
    contract H on the partition axis with no device transposes
  - scores computed in natural (q, keys) layout; mask folded into PSUM
    via an identity-matmul accumulate before the score matmuls
  - softmax row sums come free from activation(Exp, accum_out=...)
  - attn written to DRAM in natural layout (bf16, host upcasts)
  - attn transposed on PE (128x128 tiles) for the ctx matmul
  - Wf/out computed with ctxT/decT as lhsT chunks, residual+bias on DVE,
    LayerNorm stats via Square(accum_out)+reduce
Matmuls run as float32r (full-rate fp32) where precision matters.
"""

import os
import sys
import numpy as np

sys.path.insert(0, "/opt/trn_rl_repo")

import ml_dtypes

B, S, H, NH = 4, 2048, 1024, 4
D = H // NH          # 256
Q = 1024             # query rows per core
QC = Q // 128        # 8 q chunks
KT = S // 512        # 4 key tiles
KC = S // 128        # 16 key chunks
HC = H // 128        # 8 H chunks
NEG = -1.0e9
LN_EPS = 1e-5

_BUILT = None
LAST_RESULTS = None


def _build():
    import concourse.bass as bass
    import concourse.bacc as bacc_mod
    import concourse.mybir as mybir
    import concourse.tile as tile
    from concourse.masks import make_identity
    from contextlib import ExitStack

    f32 = mybir.dt.float32
    bf16 = mybir.dt.bfloat16
    f32r = mybir.dt.float32r
    AF = mybir.ActivationFunctionType
    ALU = mybir.AluOpType

    def r(ap):
        return ap.bitcast(f32r)

    nc = bacc_mod.Bacc()

    memT = nc.declare_dram_parameter("memT", [H, S], f32r, isOutput=False)
    decT = nc.declare_dram_parameter("decT", [H, Q], f32r, isOutput=False)
    decTh = nc.declare_dram_parameter("decTh", [H, Q], bf16, isOutput=False)
    dec = nc.declare_dram_parameter("dec", [Q, H], f32, isOutput=False)
    maskq = nc.declare_dram_parameter("maskq", [Q, S], bf16, isOutput=False)
    wqT = nc.declare_dram_parameter("wqT", [H, H], f32r, isOutput=False)   # pre-scaled by 1/sqrt(D)
    wkT = nc.declare_dram_parameter("wkT", [H, H], f32r, isOutput=False)
    wvT = nc.declare_dram_parameter("wvT", [H, H], f32r, isOutput=False)
    wfT = nc.declare_dram_parameter("wfT", [2 * H, H], bf16, isOutput=False)
    qmc = nc.declare_dram_parameter("qmc", [128, QC], f32, isOutput=False)
    lnsc = nc.declare_dram_parameter("lnsc", [128, H], f32, isOutput=False)
    lnbi = nc.declare_dram_parameter("lnbi", [128, H], f32, isOutput=False)
    bfb = nc.declare_dram_parameter("bfb", [128, H], f32, isOutput=False)
    attns = nc.declare_dram_parameter("attns", [NH, Q, S], bf16, isOutput=True)
    outp = nc.declare_dram_parameter("outp", [Q, H], f32, isOutput=True)

    with tile.TileContext(nc) as tc, ExitStack() as top:
        const = top.enter_context(tc.tile_pool(name="const", bufs=1))
        p_persist = top.enter_context(tc.tile_pool(name="persist", bufs=1))
        p_small = top.enter_context(tc.tile_pool(name="small", bufs=2))
        mem_stack = top.enter_context(ExitStack())
        p_memT = mem_stack.enter_context(tc.tile_pool(name="pmemT", bufs=1))
        p_ps = top.enter_context(tc.tile_pool(name="ps", bufs=2, space="PSUM"))
        p_pst = top.enter_context(tc.tile_pool(name="pst", bufs=2, space="PSUM"))
        p_psc = top.enter_context(tc.tile_pool(name="psc", bufs=2, space="PSUM"))

        idb = const.tile([128, 128], bf16)
        make_identity(nc, idb)
        qm_sb = const.tile([128, QC], f32)
        nc.sync.dma_start(qm_sb[:], qmc[:])
        ctxT_sb = p_persist.tile([128, HC, Q], bf16)

        memT_sb = p_memT.tile([128, HC, S], f32r)
        nc.sync.dma_start(memT_sb[:], memT.rearrange("(c p) s -> p c s", p=128))

        for h in range(NH):
            with ExitStack() as hs:
                p_kv = hs.enter_context(tc.tile_pool(name=f"kv{h}", bufs=1))
                p_w = hs.enter_context(tc.tile_pool(name=f"w{h}", bufs=2))
                p_wv = hs.enter_context(tc.tile_pool(name=f"wv{h}", bufs=1))
                p_dt = hs.enter_context(tc.tile_pool(name=f"dt{h}", bufs=3))

                kT_sb = p_kv.tile([128, 2, S], f32r, tag="kT")
                qT_sb = p_kv.tile([128, 2, Q], f32r, tag="qT")
                v_sb = p_kv.tile([128, KC, D], bf16, tag="v")

                # ---- k projection: kT[dh] = (Wk_h @ mem.T)[dh*128:...] ----
                for dh in range(2):
                    wk = p_w.tile([128, HC, 128], f32r, tag="w128")
                    nc.sync.dma_start(
                        wk[:],
                        wkT[:, h * D + dh * 128 : h * D + (dh + 1) * 128].rearrange(
                            "(c p) m -> p c m", p=128
                        ),
                    )
                    for half in range(2):
                        ps = p_ps.tile([128, 1024], f32, tag="ps")
                        for nt in range(2):
                            ksl = slice(half * 1024 + nt * 512, half * 1024 + (nt + 1) * 512)
                            for hc in range(HC):
                                nc.tensor.matmul(
                                    ps[:, nt * 512 : (nt + 1) * 512],
                                    r(wk[:, hc, :]),
                                    r(memT_sb[:, hc, ksl]),
                                    start=(hc == 0),
                                    stop=(hc == HC - 1),
                                )
                        nc.any.tensor_copy(
                            kT_sb[:, dh, half * 1024 : (half + 1) * 1024], ps[:]
                        )

                # ---- q projection (hc-outer, both dh psums live) ----
                wq0 = p_w.tile([128, HC, 128], f32r, tag="w128")
                nc.sync.dma_start(
                    wq0[:],
                    wqT[:, h * D : h * D + 128].rearrange("(c p) m -> p c m", p=128),
                )
                wq1 = p_w.tile([128, HC, 128], f32r, tag="w128")
                nc.sync.dma_start(
                    wq1[:],
                    wqT[:, h * D + 128 : h * D + 256].rearrange("(c p) m -> p c m", p=128),
                )
                psq = [p_ps.tile([128, 1024], f32, tag="ps", name=f"psq{dd}") for dd in range(2)]
                for hc in range(HC):
                    dt = p_dt.tile([128, Q], f32r, tag="dt")
                    nc.sync.dma_start(dt[:], decT[hc * 128 : (hc + 1) * 128, :])
                    for dh, wq in enumerate((wq0, wq1)):
                        for nt in range(2):
                            nc.tensor.matmul(
                                psq[dh][:, nt * 512 : (nt + 1) * 512],
                                r(wq[:, hc, :]),
                                r(dt[:, nt * 512 : (nt + 1) * 512]),
                                start=(hc == 0),
                                stop=(hc == HC - 1),
                            )
                for dh in range(2):
                    nc.any.tensor_copy(qT_sb[:, dh, :], psq[dh][:])

                # ---- v projection: v[kc] = mem[kc] @ Wv_h.T ----
                wv = p_wv.tile([128, HC, D], f32r, tag="w256")
                nc.sync.dma_start(
                    wv[:],
                    wvT[:, h * D : (h + 1) * D].rearrange("(c p) m -> p c m", p=128),
                )
                for kc in range(KC):
                    psv = p_psc.tile([128, 512], f32, tag="psc")
                    for hc in range(HC):
                        nc.tensor.matmul(
                            psv[:, :D],
                            r(memT_sb[:, hc, kc * 128 : (kc + 1) * 128]),
                            r(wv[:, hc, :]),
                            start=(hc == 0),
                            stop=(hc == HC - 1),
                        )
                    nc.any.tensor_copy(v_sb[:, kc, :], psv[:, :D])

                # ---- attention ----
                with ExitStack() as asx:
                    p_mk = asx.enter_context(tc.tile_pool(name=f"mk{h}", bufs=3))
                    p_e = asx.enter_context(tc.tile_pool(name=f"e{h}", bufs=3))
                    p_at = asx.enter_context(tc.tile_pool(name=f"at{h}", bufs=1))

                    attnT_sb = p_at.tile([128, KC, 512], bf16, tag="attnT")
                    sst = p_small.tile([128, QC, 3], f32, tag="sst")
                    sums = sst[:, :, 0:2]
                    scal = sst[:, :, 2]

                    for qc in range(QC):
                        mk = p_mk.tile([128, S], bf16, tag="mk")
                        nc.sync.dma_start(mk[:], maskq[qc * 128 : (qc + 1) * 128, :])
                        e = p_e.tile([128, S], bf16, tag="e")
                        for g in range(2):
                            ps = p_ps.tile([128, 1024], f32, tag="ps")
                            for kt in range(2):
                                ksl = slice(g * 1024 + kt * 512, g * 1024 + (kt + 1) * 512)
                                osl = slice(kt * 512, (kt + 1) * 512)
                                nc.tensor.matmul(
                                    ps[:, osl], idb[:], mk[:, ksl],
                                    start=True, stop=False,
                                )
                                for dh in range(2):
                                    nc.tensor.matmul(
                                        ps[:, osl],
                                        r(qT_sb[:, dh, qc * 128 : (qc + 1) * 128]),
                                        r(kT_sb[:, dh, ksl]),
                                        start=False,
                                        stop=(dh == 1),
                                    )
                            nc.scalar.activation(
                                e[:, g * 1024 : (g + 1) * 1024],
                                ps[:],
                                AF.Exp,
                                accum_out=sums[:, qc, g : g + 1],
                            )
                        # scale = query_mask / (sums_g0 + sums_g1)
                        nc.vector.tensor_tensor(
                            scal[:, qc : qc + 1],
                            sums[:, qc, 0:1],
                            sums[:, qc, 1:2],
                            ALU.add,
                        )
                        nc.vector.reciprocal(scal[:, qc : qc + 1], scal[:, qc : qc + 1])
                        nc.vector.tensor_tensor(
                            scal[:, qc : qc + 1],
                            scal[:, qc : qc + 1],
                            qm_sb[:, qc : qc + 1],
                            ALU.mult,
                        )
                        nc.vector.tensor_scalar_mul(e[:], e[:], scal[:, qc : qc + 1])
                        nc.sync.dma_start(attns[h, qc * 128 : (qc + 1) * 128, :], e[:])
                        # transpose e into attnT (per 128x128 tile)
                        qo = (qc % 4) * 128
                        for k4 in range(4):
                            pst = p_pst.tile([128, 512], bf16, tag="pst")
                            for j in range(4):
                                kc = k4 * 4 + j
                                nc.tensor.transpose(
                                    pst[:, j * 128 : (j + 1) * 128],
                                    e[:, kc * 128 : (kc + 1) * 128],
                                    idb[:],
                                )
                            nc.any.tensor_copy(
                                attnT_sb[:, k4 * 4 : (k4 + 1) * 4, qo : qo + 128],
                                pst[:].rearrange("p (j q) -> p j q", j=4),
                            )
                        if qc % 4 == 3:
                            qt = qc // 4
                            for dh in range(2):
                                psc = p_psc.tile([128, 512], f32, tag="psc")
                                for kc in range(KC):
                                    nc.tensor.matmul(
                                        psc[:],
                                        v_sb[:, kc, dh * 128 : (dh + 1) * 128],
                                        attnT_sb[:, kc, :],
                                        start=(kc == 0),
                                        stop=(kc == KC - 1),
                                    )
                                nc.any.tensor_copy(
                                    ctxT_sb[:, h * 2 + dh, qt * 512 : (qt + 1) * 512],
                                    psc[:],
                                )

        # ---- epilogue: Wf, bias, residual, LayerNorm ----
        mem_stack.close()
        with ExitStack() as es:
            p_ep = es.enter_context(tc.tile_pool(name="pep", bufs=1))
            p_o = es.enter_context(tc.tile_pool(name="po", bufs=2))
            p_dc = es.enter_context(tc.tile_pool(name="pdc", bufs=3))
            p_st = es.enter_context(tc.tile_pool(name="pstat", bufs=2))
            lnsc_sb = p_ep.tile([128, H], f32)
            nc.sync.dma_start(lnsc_sb[:], lnsc[:])
            lnbi_sb = p_ep.tile([128, H], f32)
            nc.sync.dma_start(lnbi_sb[:], lnbi[:])
            bfb_sb = p_ep.tile([128, H], f32)
            nc.sync.dma_start(bfb_sb[:], bfb[:])
            decTh_sb = p_ep.tile([128, HC, Q], bf16)
            nc.sync.dma_start(decTh_sb[:], decTh.rearrange("(c p) q -> p c q", p=128))
            wfT_sb = p_ep.tile([128, 2 * HC, H], bf16)
            nc.sync.dma_start(wfT_sb[:], wfT.rearrange("(c p) n -> p c n", p=128))
            for rc in range(QC):
                rsl = slice(rc * 128, (rc + 1) * 128)
                dc = p_dc.tile([128, H], f32, tag="dc")
                nc.sync.dma_start(dc[:], dec[rsl, :])
                pso = p_ps.tile([128, 1024], f32, tag="ps")
                for nt in range(2):
                    osl = slice(nt * 512, (nt + 1) * 512)
                    for fc in range(2 * HC):
                        lhsT = (
                            decTh_sb[:, fc, rsl]
                            if fc < HC
                            else ctxT_sb[:, fc - HC, rsl]
                        )
                        nc.tensor.matmul(
                            pso[:, osl],
                            lhsT,
                            wfT_sb[:, fc, osl],
                            start=(fc == 0),
                            stop=(fc == 2 * HC - 1),
                        )
                o = p_o.tile([128, H], f32, tag="o")
                osq = p_o.tile([128, H], f32, tag="osq")
                st = p_st.tile([128, 8], f32, tag="st")
                nc.vector.tensor_tensor(o[:], pso[:], dc[:], ALU.add)
                nc.vector.tensor_tensor(o[:], o[:], bfb_sb[:], ALU.add)
                # stats: s1 = sum(x), s2 = sum(x^2)
                nc.scalar.activation(
                    osq[:], o[:], AF.Square, accum_out=st[:, 1:2]
                )
                nc.vector.tensor_reduce(st[:, 0:1], o[:], mybir.AxisListType.X, ALU.add)
                nc.vector.tensor_scalar_mul(st[:, 2:3], st[:, 0:1], 1.0 / H)   # mean
                nc.vector.tensor_scalar_mul(st[:, 3:4], st[:, 1:2], 1.0 / H)   # E[x^2]
                nc.vector.tensor_tensor(st[:, 4:5], st[:, 2:3], st[:, 2:3], ALU.mult)
                nc.vector.tensor_tensor(st[:, 5:6], st[:, 3:4], st[:, 4:5], ALU.subtract)
                nc.vector.tensor_scalar_add(st[:, 6:7], st[:, 5:6], LN_EPS)
                nc.scalar.activation(st[:, 6:7], st[:, 6:7], AF.Sqrt)
                nc.vector.reciprocal(st[:, 7:8], st[:, 6:7])
                nc.vector.tensor_scalar(
                    o[:], o[:], st[:, 2:3], st[:, 7:8], ALU.subtract, ALU.mult
                )
                nc.vector.tensor_tensor(o[:], o[:], lnsc_sb[:], ALU.mult)
                nc.vector.tensor_tensor(o[:], o[:], lnbi_sb[:], ALU.add)
                nc.sync.dma_start(outp[rsl, :], o[:])

    nc.finalize()
    return nc


def _get_built():
    global _BUILT
    if _BUILT is None:
        _BUILT = _build()
    return _BUILT


def kernel(memory, decoder_input, mask, query_mask, Wk, Wv, Wq, Wf, bf, ln_scale,
           ln_bias):
    global LAST_RESULTS
    from concourse.bass_utils import run_bass_kernel_spmd

    memory = np.asarray(memory, np.float32)
    decoder_input = np.asarray(decoder_input, np.float32)
    mask = np.asarray(mask)
    query_mask = np.asarray(query_mask, np.float32)
    Wk = np.asarray(Wk, np.float32)
    Wv = np.asarray(Wv, np.float32)
    Wq = np.asarray(Wq, np.float32)
    Wf = np.asarray(Wf, np.float32)
    bf16 = ml_dtypes.bfloat16

    wqT = np.ascontiguousarray(Wq.T) * np.float32(1.0 / np.sqrt(D))
    wkT = np.ascontiguousarray(Wk.T)
    wvT = np.ascontiguousarray(Wv.T)
    wfT = np.ascontiguousarray(Wf.T).astype(bf16)
    lnsc_b = np.tile(np.asarray(ln_scale, np.float32)[None, :], (128, 1))
    lnbi_b = np.tile(np.asarray(ln_bias, np.float32)[None, :], (128, 1))
    bfb_b = np.tile(np.asarray(bf, np.float32)[None, :], (128, 1))

    in_maps = []
    for c in range(8):
        b, rb = c // 2, c % 2
        qsl = slice(rb * Q, (rb + 1) * Q)
        memT = np.ascontiguousarray(memory[b].T)
        decT_full = np.ascontiguousarray(decoder_input[b].T)
        decT = np.ascontiguousarray(decT_full[:, qsl])
        in_maps.append({
            "memT": memT,
            "decT": decT,
            "decTh": decT.astype(bf16),
            "dec": np.ascontiguousarray(decoder_input[b, qsl]),
            "maskq": (mask[b, qsl].astype(np.float32) * np.float32(NEG)).astype(bf16),
            "wqT": wqT, "wkT": wkT, "wvT": wvT, "wfT": wfT,
            "qmc": np.ascontiguousarray(query_mask[b, qsl].reshape(QC, 128).T),
            "lnsc": lnsc_b, "lnbi": lnbi_b, "bfb": bfb_b,
        })

    nc = _get_built()
    LAST_RESULTS = run_bass_kernel_spmd(nc, in_maps, core_ids=list(range(8)))
    res = LAST_RESULTS.results

    out = np.empty((B, S, H), np.float32)
    attns = np.empty((B, NH, S, S), np.float32)
    for c in range(8):
        b, rb = c // 2, c % 2
        qsl = slice(rb * Q, (rb + 1) * Q)
        out[b, qsl] = res[c]["outp"]
        # reference attns[i,j] = attn[head=i, batch=j] (torch .view regroup)
        attns[:, b, qsl, :] = res[c]["attns"].astype(np.float32)
    return out, attns


# revision 24
# speedup vs baseline: 1.2947x; 1.2947x over previous
"""Trainium2 Bass kernel for nn_Attention_6932077216322.

Multi-head cross-attention + concat-projection + residual + LayerNorm,
returning (out, attns) like the reference.

Sharding: pure data-parallel over (batch, query-row-block): 8 cores,
core c handles batch c//2, query rows (c%2)*1024 .. +1024, all 4 heads,
all 2048 keys. Zero collectives; k/v projections are duplicated between
the two cores of a batch (22% extra PE, beats 2-rank collective cost).

Layout strategy (per core):
  - host pre-transposes memory/decoder_input to (H, S) so projections
    contract H on the partition axis with no device transposes
  - scores computed in natural (q, keys) layout; mask folded into PSUM
    via an identity-matmul accumulate before the score matmuls
  - softmax row sums come free from activation(Exp, accum_out=...)
  - attn written to DRAM in natural layout (bf16, host upcasts)
  - attn transposed on PE (128x128 tiles) for the ctx matmul
  - Wf/out computed with ctxT/decT as lhsT chunks, residual+bias on DVE,
    LayerNorm stats via Square(accum_out)+reduce
Matmuls run as float32r (full-rate fp32) where precision matters.
"""

import os
import sys
import numpy as np

sys.path.insert(0, "/opt/trn_rl_repo")

import ml_dtypes

B, S, H, NH = 4, 2048, 1024, 4
D = H // NH          # 256
Q = 1024             # query rows per core
QC = Q // 128        # 8 q chunks
KT = S // 512        # 4 key tiles
KC = S // 128        # 16 key chunks
HC = H // 128        # 8 H chunks
NEG = -1.0e9
LN_EPS = 1e-5

_BUILT = None
LAST_RESULTS = None


def _build():
    import concourse.bass as bass
    import concourse.bacc as bacc_mod
    import concourse.mybir as mybir
    import concourse.tile as tile
    from concourse.masks import make_identity
    from contextlib import ExitStack

    f32 = mybir.dt.float32
    bf16 = mybir.dt.bfloat16
    f32r = mybir.dt.float32r
    fp8 = mybir.dt.float8e4
    AF = mybir.ActivationFunctionType
    ALU = mybir.AluOpType

    def r(ap):
        return ap.bitcast(f32r)

    nc = bacc_mod.Bacc()

    memT = nc.declare_dram_parameter("memT", [H, S], f32r, isOutput=False)
    decT = nc.declare_dram_parameter("decT", [H, Q], f32r, isOutput=False)
    decTh = nc.declare_dram_parameter("decTh", [H, Q], bf16, isOutput=False)
    dec = nc.declare_dram_parameter("dec", [Q, H], f32, isOutput=False)
    maskq = nc.declare_dram_parameter("maskq", [Q, S], fp8, isOutput=False)
    wqT = nc.declare_dram_parameter("wqT", [H, H], f32r, isOutput=False)   # pre-scaled by 1/sqrt(D)
    wkT = nc.declare_dram_parameter("wkT", [H, H], f32r, isOutput=False)
    wvT = nc.declare_dram_parameter("wvT", [H, H], f32r, isOutput=False)
    wfT = nc.declare_dram_parameter("wfT", [2 * H, H], bf16, isOutput=False)
    qmc = nc.declare_dram_parameter("qmc", [128, QC], f32, isOutput=False)
    lnsc = nc.declare_dram_parameter("lnsc", [128, H], f32, isOutput=False)
    lnbi = nc.declare_dram_parameter("lnbi", [128, H], f32, isOutput=False)
    bfb = nc.declare_dram_parameter("bfb", [128, H], f32, isOutput=False)
    attns_h = [
        nc.declare_dram_parameter(f"attns{i}", [Q, S], bf16, isOutput=True)
        for i in range(NH)
    ]
    outp = nc.declare_dram_parameter("outp", [Q, H], f32, isOutput=True)

    with tile.TileContext(nc) as tc, ExitStack() as top:
        const = top.enter_context(tc.tile_pool(name="const", bufs=1))
        p_persist = top.enter_context(tc.tile_pool(name="persist", bufs=1))
        p_small = top.enter_context(tc.tile_pool(name="small", bufs=1))
        mem_stack = top.enter_context(ExitStack())
        p_memT = mem_stack.enter_context(tc.tile_pool(name="pmemT", bufs=1))
        p_wv = mem_stack.enter_context(tc.tile_pool(name="pwv", bufs=1))
        p_ps = top.enter_context(tc.tile_pool(name="ps", bufs=2, space="PSUM"))
        p_pst = top.enter_context(tc.tile_pool(name="pst", bufs=2, space="PSUM"))
        p_psc = top.enter_context(tc.tile_pool(name="psc", bufs=2, space="PSUM"))


        ctxT_sb = p_persist.tile([128, HC, Q], bf16)

        idb = const.tile([128, 128], bf16)
        make_identity(nc, idb)
        id8 = const.tile([128, 128], fp8)
        make_identity(nc, id8)

        memT_sb = p_memT.tile([128, HC, S], f32r)
        memT_r = memT.rearrange("(c p) s -> p c s", p=128)
        for hc in range(HC):
            nc.sync.dma_start(memT_sb[:, hc, :], memT_r[:, hc, :])

        for h in range(NH):
            with ExitStack() as hs:
                p_kv = hs.enter_context(tc.tile_pool(name=f"kv{h}", bufs=1))
                p_q1 = hs.enter_context(tc.tile_pool(name=f"q1{h}", bufs=1))
                v_sb = p_q1.tile([128, KC, D], bf16, tag="v", name=f"v{h}")
                p_w = hs.enter_context(tc.tile_pool(name=f"w{h}", bufs=2))
                p_dt = hs.enter_context(tc.tile_pool(name=f"dt{h}", bufs=2))
                kT_sb = p_kv.tile([128, 2, S], f32r, tag="kT")
                qT_sb = p_q1.tile([128, 2, Q], f32r, tag="qT")

                # ---- k projection: kT[dh] = (Wk_h @ mem.T)[dh*128:...] ----
                for dh in range(2):
                    wk = p_w.tile([128, HC, 128], f32r, tag="w128")
                    nc.gpsimd.dma_start(
                        wk[:],
                        wkT[:, h * D + dh * 128 : h * D + (dh + 1) * 128].rearrange(
                            "(c p) m -> p c m", p=128
                        ),
                    )
                    for half in range(2):
                        ps = p_ps.tile([128, 1024], f32, tag="ps")
                        for nt in range(2):
                            ksl = slice(half * 1024 + nt * 512, half * 1024 + (nt + 1) * 512)
                            for hc in range(HC):
                                nc.tensor.matmul(
                                    ps[:, nt * 512 : (nt + 1) * 512],
                                    r(wk[:, hc, :]),
                                    r(memT_sb[:, hc, ksl]),
                                    start=(hc == 0),
                                    stop=(hc == HC - 1),
                                )
                        nc.any.tensor_copy(
                            kT_sb[:, dh, half * 1024 : (half + 1) * 1024], ps[:]
                        )

                # ---- q projection (hc-outer, both dh psums live) ----
                wq0 = p_w.tile([128, HC, 128], f32r, tag="w128")
                nc.gpsimd.dma_start(
                    wq0[:],
                    wqT[:, h * D : h * D + 128].rearrange("(c p) m -> p c m", p=128),
                )
                wq1 = p_w.tile([128, HC, 128], f32r, tag="w128")
                nc.gpsimd.dma_start(
                    wq1[:],
                    wqT[:, h * D + 128 : h * D + 256].rearrange("(c p) m -> p c m", p=128),
                )
                psq = [p_ps.tile([128, 1024], f32, tag="ps", name=f"psq{dd}") for dd in range(2)]
                for hc in range(HC):
                    dt = p_dt.tile([128, Q], f32r, tag="dt")
                    nc.sync.dma_start(dt[:], decT[hc * 128 : (hc + 1) * 128, :])
                    for dh, wq in enumerate((wq0, wq1)):
                        for nt in range(2):
                            nc.tensor.matmul(
                                psq[dh][:, nt * 512 : (nt + 1) * 512],
                                r(wq[:, hc, :]),
                                r(dt[:, nt * 512 : (nt + 1) * 512]),
                                start=(hc == 0),
                                stop=(hc == HC - 1),
                            )
                for dh in range(2):
                    nc.any.tensor_copy(qT_sb[:, dh, :], psq[dh][:])

                # ---- v projection: v[kc] = mem[kc] @ Wv_h.T ----
                wv = p_wv.tile([128, HC, D], f32r, tag="w256")
                nc.gpsimd.dma_start(
                    wv[:],
                    wvT[:, h * D : (h + 1) * D].rearrange("(c p) m -> p c m", p=128),
                )
                for kc in range(KC):
                    psv = p_psc.tile([128, 512], f32, tag="psc")
                    for hc in range(HC):
                        nc.tensor.matmul(
                            psv[:, :D],
                            r(memT_sb[:, hc, kc * 128 : (kc + 1) * 128]),
                            r(wv[:, hc, :]),
                            start=(hc == 0),
                            stop=(hc == HC - 1),
                        )
                    nc.any.tensor_copy(v_sb[:, kc, :], psv[:, :D])

                # ---- attention ----
                with ExitStack() as asx:
                    p_mk = asx.enter_context(tc.tile_pool(name=f"mk{h}", bufs=2))
                    p_e = asx.enter_context(tc.tile_pool(name=f"e{h}", bufs=3))
                    p_at = asx.enter_context(tc.tile_pool(name=f"at{h}", bufs=1))

                    attnT_sb = p_at.tile([128, KC, 512], bf16, tag="attnT")
                    sst = p_small.tile([128, QC, 4], f32, tag="sst")
                    sums = sst[:, :, 0:2]
                    scal = sst[:, :, 2]
                    qm_sb = sst[:, :, 3]
                    nc.gpsimd.dma_start(qm_sb[:], qmc[:])

                    for qc in range(QC):
                        mk = p_mk.tile([128, S], fp8, tag="mk")
                        nc.gpsimd.dma_start(mk[:], maskq[qc * 128 : (qc + 1) * 128, :])
                        e = p_e.tile([128, S], bf16, tag="e")
                        for g in range(2):
                            ps = p_ps.tile([128, 1024], f32, tag="ps")
                            for kt in range(2):
                                ksl = slice(g * 1024 + kt * 512, g * 1024 + (kt + 1) * 512)
                                osl = slice(kt * 512, (kt + 1) * 512)
                                nc.tensor.matmul(
                                    ps[:, osl], id8[:], mk[:, ksl],
                                    start=True, stop=False,
                                )
                                for dh in range(2):
                                    nc.tensor.matmul(
                                        ps[:, osl],
                                        r(qT_sb[:, dh, qc * 128 : (qc + 1) * 128]),
                                        r(kT_sb[:, dh, ksl]),
                                        start=False,
                                        stop=(dh == 1),
                                    )
                            nc.scalar.activation(
                                e[:, g * 1024 : (g + 1) * 1024],
                                ps[:],
                                AF.Exp,
                                accum_out=sums[:, qc, g : g + 1],
                            )
                        # scale = query_mask / (sums_g0 + sums_g1)
                        nc.vector.tensor_tensor(
                            scal[:, qc : qc + 1],
                            sums[:, qc, 0:1],
                            sums[:, qc, 1:2],
                            ALU.add,
                        )
                        nc.vector.reciprocal(scal[:, qc : qc + 1], scal[:, qc : qc + 1])
                        nc.vector.tensor_tensor(
                            scal[:, qc : qc + 1],
                            scal[:, qc : qc + 1],
                            qm_sb[:, qc : qc + 1] if False else sst[:, qc, 3:4],
                            ALU.mult,
                        )
                        nc.vector.tensor_scalar_mul(e[:], e[:], scal[:, qc : qc + 1])
                        nc.sync.dma_start(attns_h[h][qc * 128 : (qc + 1) * 128, :], e[:])
                        qo = (qc % 4) * 128
                        for k4 in range(4):
                            pst = p_pst.tile([128, 512], bf16, tag="pst")
                            for j in range(4):
                                kc = k4 * 4 + j
                                nc.tensor.transpose(
                                    pst[:, j * 128 : (j + 1) * 128],
                                    e[:, kc * 128 : (kc + 1) * 128],
                                    idb[:],
                                )
                            nc.any.tensor_copy(
                                attnT_sb[:, k4 * 4 : (k4 + 1) * 4, qo : qo + 128],
                                pst[:].rearrange("p (j q) -> p j q", j=4),
                            )
                        if qc % 4 == 3:
                            qt = qc // 4
                            for dh in range(2):
                                psc = p_psc.tile([128, 512], f32, tag="psc")
                                for kc in range(KC):
                                    nc.tensor.matmul(
                                        psc[:],
                                        v_sb[:, kc, dh * 128 : (dh + 1) * 128],
                                        attnT_sb[:, kc, :],
                                        start=(kc == 0),
                                        stop=(kc == KC - 1),
                                    )
                                nc.any.tensor_copy(
                                    ctxT_sb[:, h * 2 + dh, qt * 512 : (qt + 1) * 512],
                                    psc[:],
                                )

        # ---- epilogue: Wf, bias, residual, LayerNorm ----
        mem_stack.close()
        with ExitStack() as es:
            p_ep = es.enter_context(tc.tile_pool(name="pep", bufs=1))
            p_o = es.enter_context(tc.tile_pool(name="po", bufs=2))
            p_dc = es.enter_context(tc.tile_pool(name="pdc", bufs=3))
            p_st = es.enter_context(tc.tile_pool(name="pstat", bufs=2))
            lnsc_sb = p_ep.tile([128, H], f32)
            nc.sync.dma_start(lnsc_sb[:], lnsc[:])
            lnbi_sb = p_ep.tile([128, H], f32)
            nc.sync.dma_start(lnbi_sb[:], lnbi[:])
            bfb_sb = p_ep.tile([128, H], f32)
            nc.sync.dma_start(bfb_sb[:], bfb[:])
            decTh_sb = p_ep.tile([128, HC, Q], bf16)
            nc.sync.dma_start(decTh_sb[:], decTh.rearrange("(c p) q -> p c q", p=128))
            wfT_sb = p_ep.tile([128, 2 * HC, H], bf16)
            nc.sync.dma_start(wfT_sb[:], wfT.rearrange("(c p) n -> p c n", p=128))
            for rc in range(QC):
                rsl = slice(rc * 128, (rc + 1) * 128)
                dc = p_dc.tile([128, H], f32, tag="dc")
                nc.sync.dma_start(dc[:], dec[rsl, :])
                pso = p_ps.tile([128, 1024], f32, tag="ps")
                for nt in range(2):
                    osl = slice(nt * 512, (nt + 1) * 512)
                    for fc in range(2 * HC):
                        lhsT = (
                            decTh_sb[:, fc, rsl]
                            if fc < HC
                            else ctxT_sb[:, fc - HC, rsl]
                        )
                        nc.tensor.matmul(
                            pso[:, osl],
                            lhsT,
                            wfT_sb[:, fc, osl],
                            start=(fc == 0),
                            stop=(fc == 2 * HC - 1),
                        )
                o = p_o.tile([128, H], f32, tag="o")
                osq = p_o.tile([128, H], f32, tag="osq")
                st = p_st.tile([128, 8], f32, tag="st")
                nc.vector.tensor_tensor(o[:], pso[:], dc[:], ALU.add)
                nc.vector.tensor_tensor(o[:], o[:], bfb_sb[:], ALU.add)
                # stats: s1 = sum(x), s2 = sum(x^2)
                nc.scalar.activation(
                    osq[:], o[:], AF.Square, accum_out=st[:, 1:2]
                )
                nc.vector.tensor_reduce(st[:, 0:1], o[:], mybir.AxisListType.X, ALU.add)
                nc.vector.tensor_scalar_mul(st[:, 2:3], st[:, 0:1], 1.0 / H)   # mean
                nc.vector.tensor_scalar_mul(st[:, 3:4], st[:, 1:2], 1.0 / H)   # E[x^2]
                nc.vector.tensor_tensor(st[:, 4:5], st[:, 2:3], st[:, 2:3], ALU.mult)
                nc.vector.tensor_tensor(st[:, 5:6], st[:, 3:4], st[:, 4:5], ALU.subtract)
                nc.vector.tensor_scalar_add(st[:, 6:7], st[:, 5:6], LN_EPS)
                nc.scalar.activation(st[:, 6:7], st[:, 6:7], AF.Sqrt)
                nc.vector.reciprocal(st[:, 7:8], st[:, 6:7])
                nc.vector.tensor_scalar(
                    o[:], o[:], st[:, 2:3], st[:, 7:8], ALU.subtract, ALU.mult
                )
                nc.vector.tensor_tensor(o[:], o[:], lnsc_sb[:], ALU.mult)
                nc.vector.tensor_tensor(o[:], o[:], lnbi_sb[:], ALU.add)
                nc.sync.dma_start(outp[rsl, :], o[:])

    nc.finalize()
    return nc


def _get_built():
    global _BUILT
    if _BUILT is None:
        _BUILT = _build()
    return _BUILT


def kernel(memory, decoder_input, mask, query_mask, Wk, Wv, Wq, Wf, bf, ln_scale,
           ln_bias):
    global LAST_RESULTS
    from concourse.bass_utils import run_bass_kernel_spmd

    memory = np.asarray(memory, np.float32)
    decoder_input = np.asarray(decoder_input, np.float32)
    mask = np.asarray(mask)
    query_mask = np.asarray(query_mask, np.float32)
    Wk = np.asarray(Wk, np.float32)
    Wv = np.asarray(Wv, np.float32)
    Wq = np.asarray(Wq, np.float32)
    Wf = np.asarray(Wf, np.float32)
    bf16 = ml_dtypes.bfloat16

    wqT = np.ascontiguousarray(Wq.T) * np.float32(1.0 / np.sqrt(D))
    wkT = np.ascontiguousarray(Wk.T)
    wvT = np.ascontiguousarray(Wv.T)
    wfT = np.ascontiguousarray(Wf.T).astype(bf16)
    lnsc_b = np.tile(np.asarray(ln_scale, np.float32)[None, :], (128, 1))
    lnbi_b = np.tile(np.asarray(ln_bias, np.float32)[None, :], (128, 1))
    bfb_b = np.tile(np.asarray(bf, np.float32)[None, :], (128, 1))

    in_maps = []
    for c in range(8):
        b, rb = c // 2, c % 2
        qsl = slice(rb * Q, (rb + 1) * Q)
        memT = np.ascontiguousarray(memory[b].T)
        decT_full = np.ascontiguousarray(decoder_input[b].T)
        decT = np.ascontiguousarray(decT_full[:, qsl])
        in_maps.append({
            "memT": memT,
            "decT": decT,
            "decTh": decT.astype(bf16),
            "dec": np.ascontiguousarray(decoder_input[b, qsl]),
            "maskq": (mask[b, qsl].astype(np.float32) * np.float32(-192.0)).astype(ml_dtypes.float8_e4m3),
            "wqT": wqT, "wkT": wkT, "wvT": wvT, "wfT": wfT,
            "qmc": np.ascontiguousarray(query_mask[b, qsl].reshape(QC, 128).T),
            "lnsc": lnsc_b, "lnbi": lnbi_b, "bfb": bfb_b,
        })

    nc = _get_built()
    LAST_RESULTS = run_bass_kernel_spmd(nc, in_maps, core_ids=list(range(8)))
    res = LAST_RESULTS.results

    out = np.empty((B, S, H), np.float32)
    attns = np.empty((B, NH, S, S), np.float32)
    for c in range(8):
        b, rb = c // 2, c % 2
        qsl = slice(rb * Q, (rb + 1) * Q)
        out[b, qsl] = res[c]["outp"]
        # reference attns[i,j] = attn[head=i, batch=j] (torch .view regroup)
        for hh in range(NH):
            attns[hh, b, qsl, :] = res[c][f"attns{hh}"].astype(np.float32)
    return out, attns


# revision 27
# speedup vs baseline: 34307.9168x; 26498.1351x over previous
"""Trainium2 Bass kernel for nn_Attention_6932077216322.

Multi-head cross-attention + concat-projection + residual + LayerNorm,
returning (out, attns) like the reference.

Sharding: pure data-parallel over (batch, query-row-block): 8 cores,
core c handles batch c//2, query rows (c%2)*1024 .. +1024, all 4 heads,
all 2048 keys. Zero collectives; k/v projections are duplicated between
the two cores of a batch (22% extra PE, beats 2-rank collective cost).

Layout strategy (per core):
  - host pre-transposes memory/decoder_input to (H, S) so projections
    contract H on the partition axis with no device transposes
  - scores computed in natural (q, keys) layout; the additive mask
    (0 / -192, stored fp8e4m3) is folded into PSUM via an fp8
    identity-matmul accumulate before the fp32r score matmuls
  - softmax row sums come free from activation(Exp, accum_out=...)
  - unsafe softmax (no max subtraction): |scores| <~ 10, exp is safe in
    f32, and masked lanes underflow to exactly 0
  - attn written to DRAM in natural layout (bf16, host upcasts)
  - attn transposed on PE (128x128 tiles) for the ctx matmul
  - Wf/out computed with ctxT/decTh (bf16) as lhsT chunks + wfT rhs,
    residual+bias on DVE, LayerNorm stats via Square(accum_out)+reduce
  - bulk DMA striped across the sync+gpsimd queues; small latency-
    critical streams (weights, mask) ride the gpsimd queue
Matmuls run as float32r (full-rate fp32) where precision matters;
CoreSim cost model: ~390 us per core (8 cores run concurrently).
"""

import os
import sys
import numpy as np

sys.path.insert(0, "/opt/trn_rl_repo")

import ml_dtypes

B, S, H, NH = 4, 2048, 1024, 4
D = H // NH          # 256
Q = 1024             # query rows per core
QC = Q // 128        # 8 q chunks
KT = S // 512        # 4 key tiles
KC = S // 128        # 16 key chunks
HC = H // 128        # 8 H chunks
NEG = -1.0e9
LN_EPS = 1e-5

_BUILT = None
LAST_RESULTS = None


def _build():
    import concourse.bass as bass
    import concourse.bacc as bacc_mod
    import concourse.mybir as mybir
    import concourse.tile as tile
    from concourse.masks import make_identity
    from contextlib import ExitStack

    f32 = mybir.dt.float32
    bf16 = mybir.dt.bfloat16
    f32r = mybir.dt.float32r
    fp8 = mybir.dt.float8e4
    AF = mybir.ActivationFunctionType
    ALU = mybir.AluOpType

    def r(ap):
        return ap.bitcast(f32r)

    nc = bacc_mod.Bacc()

    memT = nc.declare_dram_parameter("memT", [H, S], f32r, isOutput=False)
    decT = nc.declare_dram_parameter("decT", [H, Q], f32r, isOutput=False)
    decTh = nc.declare_dram_parameter("decTh", [H, Q], bf16, isOutput=False)
    dec = nc.declare_dram_parameter("dec", [Q, H], f32, isOutput=False)
    maskq = nc.declare_dram_parameter("maskq", [Q, S], fp8, isOutput=False)
    wqT = nc.declare_dram_parameter("wqT", [H, H], f32r, isOutput=False)   # pre-scaled by 1/sqrt(D)
    wkT = nc.declare_dram_parameter("wkT", [H, H], f32r, isOutput=False)
    wvT = nc.declare_dram_parameter("wvT", [H, H], f32r, isOutput=False)
    wfT = nc.declare_dram_parameter("wfT", [2 * H, H], bf16, isOutput=False)
    qmc = nc.declare_dram_parameter("qmc", [128, QC], f32, isOutput=False)
    lnsc = nc.declare_dram_parameter("lnsc", [128, H], f32, isOutput=False)
    lnbi = nc.declare_dram_parameter("lnbi", [128, H], f32, isOutput=False)
    bfb = nc.declare_dram_parameter("bfb", [128, H], f32, isOutput=False)
    attns_h = [
        nc.declare_dram_parameter(f"attns{i}", [Q, S], bf16, isOutput=True)
        for i in range(NH)
    ]
    outp = nc.declare_dram_parameter("outp", [Q, H], f32, isOutput=True)

    with tile.TileContext(nc) as tc, ExitStack() as top:
        const = top.enter_context(tc.tile_pool(name="const", bufs=1))
        p_persist = top.enter_context(tc.tile_pool(name="persist", bufs=1))
        p_small = top.enter_context(tc.tile_pool(name="small", bufs=1))
        mem_stack = top.enter_context(ExitStack())
        p_memT = mem_stack.enter_context(tc.tile_pool(name="pmemT", bufs=1))
        p_wv = mem_stack.enter_context(tc.tile_pool(name="pwv", bufs=1))
        p_ps = top.enter_context(tc.tile_pool(name="ps", bufs=2, space="PSUM"))
        p_pst = top.enter_context(tc.tile_pool(name="pst", bufs=2, space="PSUM"))
        p_psc = top.enter_context(tc.tile_pool(name="psc", bufs=2, space="PSUM"))


        ctxT_sb = p_persist.tile([128, HC, Q], bf16)

        idb = const.tile([128, 128], bf16)
        make_identity(nc, idb)
        id8 = const.tile([128, 128], fp8)
        make_identity(nc, id8)

        memT_sb = p_memT.tile([128, HC, S], f32r)
        memT_r = memT.rearrange("(c p) s -> p c s", p=128)
        for hc in range(HC):
            eng = nc.sync if hc % 2 == 0 else nc.gpsimd
            eng.dma_start(memT_sb[:, hc, :], memT_r[:, hc, :])

        for h in range(NH):
            with ExitStack() as hs:
                p_kv = hs.enter_context(tc.tile_pool(name=f"kv{h}", bufs=1))
                p_q1 = hs.enter_context(tc.tile_pool(name=f"q1{h}", bufs=1))
                v_sb = p_q1.tile([128, KC, D], bf16, tag="v", name=f"v{h}")
                p_w = hs.enter_context(tc.tile_pool(name=f"w{h}", bufs=2))
                p_dt = hs.enter_context(tc.tile_pool(name=f"dt{h}", bufs=2))
                kT_sb = p_kv.tile([128, 2, S], f32r, tag="kT")
                qT_sb = p_q1.tile([128, 2, Q], f32r, tag="qT")

                # ---- k projection: kT[dh] = (Wk_h @ mem.T)[dh*128:...] ----
                for dh in range(2):
                    wk = p_w.tile([128, HC, 128], f32r, tag="w128")
                    nc.gpsimd.dma_start(
                        wk[:],
                        wkT[:, h * D + dh * 128 : h * D + (dh + 1) * 128].rearrange(
                            "(c p) m -> p c m", p=128
                        ),
                    )
                    for half in range(2):
                        ps = p_ps.tile([128, 1024], f32, tag="ps")
                        for nt in range(2):
                            ksl = slice(half * 1024 + nt * 512, half * 1024 + (nt + 1) * 512)
                            for hc in range(HC):
                                nc.tensor.matmul(
                                    ps[:, nt * 512 : (nt + 1) * 512],
                                    r(wk[:, hc, :]),
                                    r(memT_sb[:, hc, ksl]),
                                    start=(hc == 0),
                                    stop=(hc == HC - 1),
                                )
                        nc.any.tensor_copy(
                            kT_sb[:, dh, half * 1024 : (half + 1) * 1024], ps[:]
                        )

                # ---- q projection (hc-outer, both dh psums live) ----
                wq0 = p_w.tile([128, HC, 128], f32r, tag="w128")
                nc.gpsimd.dma_start(
                    wq0[:],
                    wqT[:, h * D : h * D + 128].rearrange("(c p) m -> p c m", p=128),
                )
                wq1 = p_w.tile([128, HC, 128], f32r, tag="w128")
                nc.gpsimd.dma_start(
                    wq1[:],
                    wqT[:, h * D + 128 : h * D + 256].rearrange("(c p) m -> p c m", p=128),
                )
                psq = [p_ps.tile([128, 1024], f32, tag="ps", name=f"psq{dd}") for dd in range(2)]
                for hc in range(HC):
                    dt = p_dt.tile([128, Q], f32r, tag="dt")
                    nc.sync.dma_start(dt[:], decT[hc * 128 : (hc + 1) * 128, :])
                    for dh, wq in enumerate((wq0, wq1)):
                        for nt in range(2):
                            nc.tensor.matmul(
                                psq[dh][:, nt * 512 : (nt + 1) * 512],
                                r(wq[:, hc, :]),
                                r(dt[:, nt * 512 : (nt + 1) * 512]),
                                start=(hc == 0),
                                stop=(hc == HC - 1),
                            )
                for dh in range(2):
                    nc.any.tensor_copy(qT_sb[:, dh, :], psq[dh][:])

                # ---- v projection: v[kc] = mem[kc] @ Wv_h.T ----
                wv = p_wv.tile([128, HC, D], f32r, tag="w256")
                nc.gpsimd.dma_start(
                    wv[:],
                    wvT[:, h * D : (h + 1) * D].rearrange("(c p) m -> p c m", p=128),
                )
                for kc in range(KC):
                    psv = p_psc.tile([128, 512], f32, tag="psc")
                    for hc in range(HC):
                        nc.tensor.matmul(
                            psv[:, :D],
                            r(memT_sb[:, hc, kc * 128 : (kc + 1) * 128]),
                            r(wv[:, hc, :]),
                            start=(hc == 0),
                            stop=(hc == HC - 1),
                        )
                    nc.any.tensor_copy(v_sb[:, kc, :], psv[:, :D])

                # ---- attention ----
                with ExitStack() as asx:
                    p_mk = asx.enter_context(tc.tile_pool(name=f"mk{h}", bufs=2))
                    p_e = asx.enter_context(tc.tile_pool(name=f"e{h}", bufs=3))
                    p_at = asx.enter_context(tc.tile_pool(name=f"at{h}", bufs=1))

                    attnT_sb = p_at.tile([128, KC, 512], bf16, tag="attnT")
                    sst = p_small.tile([128, QC, 4], f32, tag="sst")
                    sums = sst[:, :, 0:2]
                    scal = sst[:, :, 2]
                    qm_sb = sst[:, :, 3]
                    nc.gpsimd.dma_start(qm_sb[:], qmc[:])

                    for qc in range(QC):
                        mk = p_mk.tile([128, S], fp8, tag="mk")
                        nc.gpsimd.dma_start(mk[:], maskq[qc * 128 : (qc + 1) * 128, :])
                        e = p_e.tile([128, S], bf16, tag="e")
                        for g in range(2):
                            ps = p_ps.tile([128, 1024], f32, tag="ps")
                            for kt in range(2):
                                ksl = slice(g * 1024 + kt * 512, g * 1024 + (kt + 1) * 512)
                                osl = slice(kt * 512, (kt + 1) * 512)
                                nc.tensor.matmul(
                                    ps[:, osl], id8[:], mk[:, ksl],
                                    start=True, stop=False,
                                )
                                for dh in range(2):
                                    nc.tensor.matmul(
                                        ps[:, osl],
                                        r(qT_sb[:, dh, qc * 128 : (qc + 1) * 128]),
                                        r(kT_sb[:, dh, ksl]),
                                        start=False,
                                        stop=(dh == 1),
                                    )
                            nc.scalar.activation(
                                e[:, g * 1024 : (g + 1) * 1024],
                                ps[:],
                                AF.Exp,
                                accum_out=sums[:, qc, g : g + 1],
                            )
                        # scale = query_mask / (sums_g0 + sums_g1)
                        nc.vector.tensor_tensor(
                            scal[:, qc : qc + 1],
                            sums[:, qc, 0:1],
                            sums[:, qc, 1:2],
                            ALU.add,
                        )
                        nc.vector.reciprocal(scal[:, qc : qc + 1], scal[:, qc : qc + 1])
                        nc.vector.tensor_tensor(
                            scal[:, qc : qc + 1],
                            scal[:, qc : qc + 1],
                            qm_sb[:, qc : qc + 1] if False else sst[:, qc, 3:4],
                            ALU.mult,
                        )
                        nc.vector.tensor_scalar_mul(e[:], e[:], scal[:, qc : qc + 1])
                        nc.sync.dma_start(attns_h[h][qc * 128 : (qc + 1) * 128, :], e[:])
                        qo = (qc % 4) * 128
                        for k4 in range(4):
                            pst = p_pst.tile([128, 512], bf16, tag="pst")
                            for j in range(4):
                                kc = k4 * 4 + j
                                nc.tensor.transpose(
                                    pst[:, j * 128 : (j + 1) * 128],
                                    e[:, kc * 128 : (kc + 1) * 128],
                                    idb[:],
                                )
                            nc.any.tensor_copy(
                                attnT_sb[:, k4 * 4 : (k4 + 1) * 4, qo : qo + 128],
                                pst[:].rearrange("p (j q) -> p j q", j=4),
                            )
                        if qc % 4 == 3:
                            qt = qc // 4
                            for dh in range(2):
                                psc = p_psc.tile([128, 512], f32, tag="psc")
                                for kc in range(KC):
                                    nc.tensor.matmul(
                                        psc[:],
                                        v_sb[:, kc, dh * 128 : (dh + 1) * 128],
                                        attnT_sb[:, kc, :],
                                        start=(kc == 0),
                                        stop=(kc == KC - 1),
                                    )
                                nc.any.tensor_copy(
                                    ctxT_sb[:, h * 2 + dh, qt * 512 : (qt + 1) * 512],
                                    psc[:],
                                )

        # ---- epilogue: Wf, bias, residual, LayerNorm ----
        mem_stack.close()
        with ExitStack() as es:
            p_ep = es.enter_context(tc.tile_pool(name="pep", bufs=1))
            p_o = es.enter_context(tc.tile_pool(name="po", bufs=2))
            p_dc = es.enter_context(tc.tile_pool(name="pdc", bufs=3))
            p_st = es.enter_context(tc.tile_pool(name="pstat", bufs=2))
            lnsc_sb = p_ep.tile([128, H], f32)
            nc.sync.dma_start(lnsc_sb[:], lnsc[:])
            lnbi_sb = p_ep.tile([128, H], f32)
            nc.sync.dma_start(lnbi_sb[:], lnbi[:])
            bfb_sb = p_ep.tile([128, H], f32)
            nc.sync.dma_start(bfb_sb[:], bfb[:])
            decTh_sb = p_ep.tile([128, HC, Q], bf16)
            nc.sync.dma_start(decTh_sb[:], decTh.rearrange("(c p) q -> p c q", p=128))
            wfT_sb = p_ep.tile([128, 2 * HC, H], bf16)
            nc.sync.dma_start(wfT_sb[:], wfT.rearrange("(c p) n -> p c n", p=128))
            for rc in range(QC):
                rsl = slice(rc * 128, (rc + 1) * 128)
                dc = p_dc.tile([128, H], f32, tag="dc")
                nc.sync.dma_start(dc[:], dec[rsl, :])
                pso = p_ps.tile([128, 1024], f32, tag="ps")
                for nt in range(2):
                    osl = slice(nt * 512, (nt + 1) * 512)
                    for fc in range(2 * HC):
                        lhsT = (
                            decTh_sb[:, fc, rsl]
                            if fc < HC
                            else ctxT_sb[:, fc - HC, rsl]
                        )
                        nc.tensor.matmul(
                            pso[:, osl],
                            lhsT,
                            wfT_sb[:, fc, osl],
                            start=(fc == 0),
                            stop=(fc == 2 * HC - 1),
                        )
                o = p_o.tile([128, H], f32, tag="o")
                osq = p_o.tile([128, H], f32, tag="osq")
                st = p_st.tile([128, 8], f32, tag="st")
                nc.vector.tensor_tensor(o[:], pso[:], dc[:], ALU.add)
                nc.vector.tensor_tensor(o[:], o[:], bfb_sb[:], ALU.add)
                # stats: s1 = sum(x), s2 = sum(x^2)
                nc.scalar.activation(
                    osq[:], o[:], AF.Square, accum_out=st[:, 1:2]
                )
                nc.vector.tensor_reduce(st[:, 0:1], o[:], mybir.AxisListType.X, ALU.add)
                nc.vector.tensor_scalar_mul(st[:, 2:3], st[:, 0:1], 1.0 / H)   # mean
                nc.vector.tensor_scalar_mul(st[:, 3:4], st[:, 1:2], 1.0 / H)   # E[x^2]
                nc.vector.tensor_tensor(st[:, 4:5], st[:, 2:3], st[:, 2:3], ALU.mult)
                nc.vector.tensor_tensor(st[:, 5:6], st[:, 3:4], st[:, 4:5], ALU.subtract)
                nc.vector.tensor_scalar_add(st[:, 6:7], st[:, 5:6], LN_EPS)
                nc.scalar.activation(st[:, 6:7], st[:, 6:7], AF.Sqrt)
                nc.vector.reciprocal(st[:, 7:8], st[:, 6:7])
                nc.vector.tensor_scalar(
                    o[:], o[:], st[:, 2:3], st[:, 7:8], ALU.subtract, ALU.mult
                )
                nc.vector.tensor_tensor(o[:], o[:], lnsc_sb[:], ALU.mult)
                nc.vector.tensor_tensor(o[:], o[:], lnbi_sb[:], ALU.add)
                nc.sync.dma_start(outp[rsl, :], o[:])

    nc.finalize()
    return nc


def _get_built():
    global _BUILT
    if _BUILT is None:
        _BUILT = _build()
    return _BUILT


def kernel(memory, decoder_input, mask, query_mask, Wk, Wv, Wq, Wf, bf, ln_scale,
           ln_bias):
    global LAST_RESULTS
    from concourse.bass_utils import run_bass_kernel_spmd

    memory = np.asarray(memory, np.float32)
    decoder_input = np.asarray(decoder_input, np.float32)
    mask = np.asarray(mask)
    query_mask = np.asarray(query_mask, np.float32)
    Wk = np.asarray(Wk, np.float32)
    Wv = np.asarray(Wv, np.float32)
    Wq = np.asarray(Wq, np.float32)
    Wf = np.asarray(Wf, np.float32)
    bf16 = ml_dtypes.bfloat16

    wqT = np.ascontiguousarray(Wq.T) * np.float32(1.0 / np.sqrt(D))
    wkT = np.ascontiguousarray(Wk.T)
    wvT = np.ascontiguousarray(Wv.T)
    wfT = np.ascontiguousarray(Wf.T).astype(bf16)
    lnsc_b = np.tile(np.asarray(ln_scale, np.float32)[None, :], (128, 1))
    lnbi_b = np.tile(np.asarray(ln_bias, np.float32)[None, :], (128, 1))
    bfb_b = np.tile(np.asarray(bf, np.float32)[None, :], (128, 1))

    in_maps = []
    for c in range(8):
        b, rb = c // 2, c % 2
        qsl = slice(rb * Q, (rb + 1) * Q)
        memT = np.ascontiguousarray(memory[b].T)
        decT_full = np.ascontiguousarray(decoder_input[b].T)
        decT = np.ascontiguousarray(decT_full[:, qsl])
        in_maps.append({
            "memT": memT,
            "decT": decT,
            "decTh": decT.astype(bf16),
            "dec": np.ascontiguousarray(decoder_input[b, qsl]),
            "maskq": (mask[b, qsl].astype(np.float32) * np.float32(-192.0)).astype(ml_dtypes.float8_e4m3),
            "wqT": wqT, "wkT": wkT, "wvT": wvT, "wfT": wfT,
            "qmc": np.ascontiguousarray(query_mask[b, qsl].reshape(QC, 128).T),
            "lnsc": lnsc_b, "lnbi": lnbi_b, "bfb": bfb_b,
        })

    nc = _get_built()
    LAST_RESULTS = run_bass_kernel_spmd(nc, in_maps, core_ids=list(range(8)))
    res = LAST_RESULTS.results

    out = np.empty((B, S, H), np.float32)
    attns = np.empty((B, NH, S, S), np.float32)
    for c in range(8):
        b, rb = c // 2, c % 2
        qsl = slice(rb * Q, (rb + 1) * Q)
        out[b, qsl] = res[c]["outp"]
        # reference attns[i,j] = attn[head=i, batch=j] (torch .view regroup)
        for hh in range(NH):
            attns[hh, b, qsl, :] = res[c][f"attns{hh}"].astype(np.float32)
    return out, attns


# revision 41
# speedup vs baseline: 35445.6360x; 1.0332x over previous
"""Trainium2 Bass kernel for nn_Attention_6932077216322.

Multi-head cross-attention + concat-projection + residual + LayerNorm,
returning (out, attns) like the reference.

Sharding: pure data-parallel over (batch, query-row-block): 8 cores,
core c handles batch c//2, query rows (c%2)*1024 .. +1024, all 4 heads,
all 2048 keys. Zero collectives; k/v projections are duplicated between
the two cores of a batch (22% extra PE, beats 2-rank collective cost).

Layout strategy (per core):
  - host pre-transposes memory/decoder_input to (H, S) so projections
    contract H on the partition axis with no device transposes
  - scores computed in natural (q, keys) layout; masking is
    MULTIPLICATIVE after exp: one DVE scalar_tensor_tensor multiplies
    exp(scores) by a 0/1 bf16 binmask (masked lanes exactly 0) and its
    accum_out yields the softmax row-sum in the same op
  - unsafe softmax (no max subtraction): |scores| <~ 10, exp safe in f32
  - attn written to DRAM in natural layout (bf16, host upcasts)
  - attn transposed on PE (128x128 tiles) for the ctx matmul
  - Wf/out computed with ctxT/decTh (bf16) as lhsT chunks + wfT rhs,
    residual+bias on DVE, LayerNorm stats via Square(accum_out)+reduce
  - bulk DMA striped across the sync+gpsimd queues; small latency-
    critical streams (weights, mask) ride the gpsimd queue
Matmuls run as float32r (full-rate fp32) where precision matters;
CoreSim cost model: ~380 us per core (8 cores run concurrently).
"""

import os
import sys
import numpy as np

sys.path.insert(0, "/opt/trn_rl_repo")

import ml_dtypes

B, S, H, NH = 4, 2048, 1024, 4
D = H // NH          # 256
Q = 1024             # query rows per core
QC = Q // 128        # 8 q chunks
KT = S // 512        # 4 key tiles
KC = S // 128        # 16 key chunks
HC = H // 128        # 8 H chunks
NEG = -1.0e9
LN_EPS = 1e-5

_BUILT = None
LAST_RESULTS = None


def _build():
    import concourse.bass as bass
    import concourse.bacc as bacc_mod
    import concourse.mybir as mybir
    import concourse.tile as tile
    from concourse.masks import make_identity
    from contextlib import ExitStack

    f32 = mybir.dt.float32
    bf16 = mybir.dt.bfloat16
    f32r = mybir.dt.float32r
    fp8 = mybir.dt.float8e4
    AF = mybir.ActivationFunctionType
    ALU = mybir.AluOpType

    def r(ap):
        return ap.bitcast(f32r)

    nc = bacc_mod.Bacc()

    memT = nc.declare_dram_parameter("memT", [H, S], f32r, isOutput=False)
    decT = nc.declare_dram_parameter("decT", [H, Q], f32r, isOutput=False)
    decTh = nc.declare_dram_parameter("decTh", [H, Q], bf16, isOutput=False)
    dec = nc.declare_dram_parameter("dec", [Q, H], f32, isOutput=False)
    maskq = nc.declare_dram_parameter("maskq", [Q, S], bf16, isOutput=False)
    wqT = nc.declare_dram_parameter("wqT", [H, H], f32r, isOutput=False)   # pre-scaled by 1/sqrt(D)
    wkT = nc.declare_dram_parameter("wkT", [H, H], f32r, isOutput=False)
    wvT = nc.declare_dram_parameter("wvT", [H, H], f32r, isOutput=False)
    wfT = nc.declare_dram_parameter("wfT", [2 * H, H], bf16, isOutput=False)
    qmc = nc.declare_dram_parameter("qmc", [128, QC], f32, isOutput=False)
    lnsc = nc.declare_dram_parameter("lnsc", [128, H], f32, isOutput=False)
    lnbi = nc.declare_dram_parameter("lnbi", [128, H], f32, isOutput=False)
    attns_h = [
        nc.declare_dram_parameter(f"attns{i}", [Q, S], bf16, isOutput=True)
        for i in range(NH)
    ]
    outp = nc.declare_dram_parameter("outp", [Q, H], f32, isOutput=True)

    with tile.TileContext(nc) as tc, ExitStack() as top:
        const = top.enter_context(tc.tile_pool(name="const", bufs=1))
        p_persist = top.enter_context(tc.tile_pool(name="persist", bufs=1))
        p_small = top.enter_context(tc.tile_pool(name="small", bufs=1))
        mem_stack = top.enter_context(ExitStack())
        p_memT = mem_stack.enter_context(tc.tile_pool(name="pmemT", bufs=1))
        p_wv = mem_stack.enter_context(tc.tile_pool(name="pwv", bufs=1))
        p_ps = top.enter_context(tc.tile_pool(name="ps", bufs=2, space="PSUM"))
        p_pst = top.enter_context(tc.tile_pool(name="pst", bufs=2, space="PSUM"))
        p_psc = top.enter_context(tc.tile_pool(name="psc", bufs=2, space="PSUM"))


        ctxT_sb = p_persist.tile([128, HC, Q], bf16)

        idb = const.tile([128, 128], bf16)
        make_identity(nc, idb)

        memT_sb = p_memT.tile([128, HC, S], f32r)
        memT_r = memT.rearrange("(c p) s -> p c s", p=128)
        for cb in range(4):
            for hc in range(HC):
                eng = nc.sync if hc % 2 == 0 else nc.gpsimd
                eng.dma_start(
                    memT_sb[:, hc, cb * 512 : (cb + 1) * 512],
                    memT_r[:, hc, cb * 512 : (cb + 1) * 512],
                )

        for h in range(NH):
            with ExitStack() as hs:
                p_kv = hs.enter_context(tc.tile_pool(name=f"kv{h}", bufs=1))
                p_q1 = hs.enter_context(tc.tile_pool(name=f"q1{h}", bufs=1))
                v_sb = p_q1.tile([128, KC, D], bf16, tag="v", name=f"v{h}")
                p_w = hs.enter_context(tc.tile_pool(name=f"w{h}", bufs=2))
                p_dt = hs.enter_context(tc.tile_pool(name=f"dt{h}", bufs=2))
                kT_sb = p_kv.tile([128, 2, S], f32r, tag="kT")
                qT_sb = p_q1.tile([128, 2, Q], f32r, tag="qT")

                # ---- k projection: kT[dh] = (Wk_h @ mem.T)[dh*128:...] ----
                wks = []
                for dh in range(2):
                    wk = p_w.tile([128, HC, 128], f32r, tag="w128", name=f"wk{dh}")
                    nc.gpsimd.dma_start(
                        wk[:],
                        wkT[:, h * D + dh * 128 : h * D + (dh + 1) * 128].rearrange(
                            "(c p) m -> p c m", p=128
                        ),
                    )
                    wks.append(wk)
                for half in range(2):
                    for dh in range(2):
                        wk = wks[dh]
                        ps = p_ps.tile([128, 1024], f32, tag="ps")
                        for nt in range(2):
                            ksl = slice(half * 1024 + nt * 512, half * 1024 + (nt + 1) * 512)
                            for hc in range(HC):
                                nc.tensor.matmul(
                                    ps[:, nt * 512 : (nt + 1) * 512],
                                    r(wk[:, hc, :]),
                                    r(memT_sb[:, hc, ksl]),
                                    start=(hc == 0),
                                    stop=(hc == HC - 1),
                                )
                        nc.any.tensor_copy(
                            kT_sb[:, dh, half * 1024 : (half + 1) * 1024], ps[:]
                        )

                # ---- q projection (hc-outer, both dh psums live) ----
                wq0 = p_w.tile([128, HC, 128], f32r, tag="w128")
                nc.gpsimd.dma_start(
                    wq0[:],
                    wqT[:, h * D : h * D + 128].rearrange("(c p) m -> p c m", p=128),
                )
                wq1 = p_w.tile([128, HC, 128], f32r, tag="w128")
                nc.gpsimd.dma_start(
                    wq1[:],
                    wqT[:, h * D + 128 : h * D + 256].rearrange("(c p) m -> p c m", p=128),
                )
                psq = [p_ps.tile([128, 1024], f32, tag="ps", name=f"psq{dd}") for dd in range(2)]
                for hc in range(HC):
                    dt = p_dt.tile([128, Q], f32r, tag="dt")
                    nc.sync.dma_start(dt[:], decT[hc * 128 : (hc + 1) * 128, :])
                    for dh, wq in enumerate((wq0, wq1)):
                        for nt in range(2):
                            nc.tensor.matmul(
                                psq[dh][:, nt * 512 : (nt + 1) * 512],
                                r(wq[:, hc, :]),
                                r(dt[:, nt * 512 : (nt + 1) * 512]),
                                start=(hc == 0),
                                stop=(hc == HC - 1),
                            )
                for dh in range(2):
                    nc.any.tensor_copy(qT_sb[:, dh, :], psq[dh][:])

                # ---- v projection: v[kc] = mem[kc] @ Wv_h.T ----
                wv = p_wv.tile([128, HC, D], f32r, tag="w256")
                nc.gpsimd.dma_start(
                    wv[:],
                    wvT[:, h * D : (h + 1) * D].rearrange("(c p) m -> p c m", p=128),
                )
                for kc in range(KC):
                    psv = p_psc.tile([128, 512], f32, tag="psc")
                    for hc in range(HC):
                        nc.tensor.matmul(
                            psv[:, :D],
                            r(memT_sb[:, hc, kc * 128 : (kc + 1) * 128]),
                            r(wv[:, hc, :]),
                            start=(hc == 0),
                            stop=(hc == HC - 1),
                        )
                    nc.any.tensor_copy(v_sb[:, kc, :], psv[:, :D])

                # ---- attention ----
                with ExitStack() as asx:
                    p_mk = asx.enter_context(tc.tile_pool(name=f"mk{h}", bufs=2))
                    p_e = asx.enter_context(tc.tile_pool(name=f"e{h}", bufs=3))
                    p_at = asx.enter_context(tc.tile_pool(name=f"at{h}", bufs=1))

                    attnT_sb = p_at.tile([128, KC, 512], bf16, tag="attnT")
                    sst = p_small.tile([128, QC, 4], f32, tag="sst")
                    sums = sst[:, :, 0:2]
                    scal = sst[:, :, 2]
                    qm_sb = sst[:, :, 3]
                    nc.gpsimd.dma_start(qm_sb[:], qmc[:])

                    for qc in range(QC):
                        mk = p_mk.tile([128, S], bf16, tag="mk")
                        nc.gpsimd.dma_start(mk[:], maskq[qc * 128 : (qc + 1) * 128, :])
                        e = p_e.tile([128, S], bf16, tag="e")
                        for g in range(2):
                            gsl = slice(g * 1024, (g + 1) * 1024)
                            ps = p_ps.tile([128, 1024], f32, tag="ps")
                            for kt in range(2):
                                ksl = slice(g * 1024 + kt * 512, g * 1024 + (kt + 1) * 512)
                                osl = slice(kt * 512, (kt + 1) * 512)
                                for dh in range(2):
                                    nc.tensor.matmul(
                                        ps[:, osl],
                                        r(qT_sb[:, dh, qc * 128 : (qc + 1) * 128]),
                                        r(kT_sb[:, dh, ksl]),
                                        start=(dh == 0),
                                        stop=(dh == 1),
                                    )
                            nc.scalar.activation(e[:, gsl], ps[:], AF.Exp)
                            # multiplicative mask (exact 0 on masked lanes)
                            # + softmax row-sum in one DVE op
                            nc.vector.scalar_tensor_tensor(
                                e[:, gsl], e[:, gsl], 1.0, mk[:, gsl],
                                ALU.mult, ALU.mult,
                                accum_out=sums[:, qc, g : g + 1],
                            )
                        # scale = query_mask / (sums_g0 + sums_g1)
                        nc.vector.tensor_tensor(
                            scal[:, qc : qc + 1],
                            sums[:, qc, 0:1],
                            sums[:, qc, 1:2],
                            ALU.add,
                        )
                        nc.vector.reciprocal(scal[:, qc : qc + 1], scal[:, qc : qc + 1])
                        nc.vector.tensor_tensor(
                            scal[:, qc : qc + 1],
                            scal[:, qc : qc + 1],
                            qm_sb[:, qc : qc + 1] if False else sst[:, qc, 3:4],
                            ALU.mult,
                        )
                        nc.vector.tensor_scalar_mul(e[:], e[:], scal[:, qc : qc + 1])
                        nc.sync.dma_start(attns_h[h][qc * 128 : (qc + 1) * 128, :], e[:])
                        qo = (qc % 4) * 128
                        for k8 in range(2):
                            pst = p_pst.tile([128, 1024], bf16, tag="pst")
                            for j in range(8):
                                kc = k8 * 8 + j
                                nc.tensor.transpose(
                                    pst[:, j * 128 : (j + 1) * 128],
                                    e[:, kc * 128 : (kc + 1) * 128],
                                    idb[:],
                                )
                            nc.any.tensor_copy(
                                attnT_sb[:, k8 * 8 : (k8 + 1) * 8, qo : qo + 128],
                                pst[:].rearrange("p (j q) -> p j q", j=8),
                            )
                        if qc % 4 == 3:
                            qt = qc // 4
                            for dh in range(2):
                                psc = p_psc.tile([128, 512], f32, tag="psc")
                                for kc in range(KC):
                                    nc.tensor.matmul(
                                        psc[:],
                                        v_sb[:, kc, dh * 128 : (dh + 1) * 128],
                                        attnT_sb[:, kc, :],
                                        start=(kc == 0),
                                        stop=(kc == KC - 1),
                                    )
                                nc.any.tensor_copy(
                                    ctxT_sb[:, h * 2 + dh, qt * 512 : (qt + 1) * 512],
                                    psc[:],
                                )

        # ---- epilogue: Wf, bias, residual, LayerNorm ----
        mem_stack.close()
        with ExitStack() as es:
            p_ep = es.enter_context(tc.tile_pool(name="pep", bufs=1))
            p_o = es.enter_context(tc.tile_pool(name="po", bufs=4))
            p_dc = es.enter_context(tc.tile_pool(name="pdc", bufs=4))
            p_st = es.enter_context(tc.tile_pool(name="pstat", bufs=4))
            lnsc_sb = p_ep.tile([128, H], f32)
            nc.sync.dma_start(lnsc_sb[:], lnsc[:])
            lnbi_sb = p_ep.tile([128, H], f32)
            nc.sync.dma_start(lnbi_sb[:], lnbi[:])
            decTh_sb = p_ep.tile([128, HC, Q], bf16)
            nc.sync.dma_start(decTh_sb[:], decTh.rearrange("(c p) q -> p c q", p=128))
            wfT_sb = p_ep.tile([128, 2 * HC, H], bf16)
            nc.sync.dma_start(wfT_sb[:], wfT.rearrange("(c p) n -> p c n", p=128))
            for rc in range(QC):
                rsl = slice(rc * 128, (rc + 1) * 128)
                dc = p_dc.tile([128, H], f32, tag="dc")
                nc.sync.dma_start(dc[:], dec[rsl, :])
                pso = p_ps.tile([128, 1024], f32, tag="ps")
                for nt in range(2):
                    osl = slice(nt * 512, (nt + 1) * 512)
                    for fc in range(2 * HC):
                        lhsT = (
                            decTh_sb[:, fc, rsl]
                            if fc < HC
                            else ctxT_sb[:, fc - HC, rsl]
                        )
                        nc.tensor.matmul(
                            pso[:, osl],
                            lhsT,
                            wfT_sb[:, fc, osl],
                            start=(fc == 0),
                            stop=(fc == 2 * HC - 1),
                        )
                o = p_o.tile([128, H], f32, tag="o")
                osq = p_o.tile([128, H], f32, tag="osq")
                st = p_st.tile([128, 8], f32, tag="st")
                # residual add + s1 = sum(x) fused
                nc.vector.scalar_tensor_tensor(
                    o[:], pso[:], 1.0, dc[:], ALU.mult, ALU.add,
                    accum_out=st[:, 0:1],
                )
                # s2 = sum(x^2)
                nc.scalar.activation(
                    osq[:], o[:], AF.Square, accum_out=st[:, 1:2]
                )
                nc.vector.tensor_scalar_mul(st[:, 2:3], st[:, 0:1], 1.0 / H)   # mean
                nc.vector.tensor_scalar_mul(st[:, 3:4], st[:, 1:2], 1.0 / H)   # E[x^2]
                nc.vector.tensor_tensor(st[:, 4:5], st[:, 2:3], st[:, 2:3], ALU.mult)
                nc.vector.tensor_tensor(st[:, 5:6], st[:, 3:4], st[:, 4:5], ALU.subtract)
                nc.vector.tensor_scalar_add(st[:, 6:7], st[:, 5:6], LN_EPS)
                nc.scalar.activation(st[:, 6:7], st[:, 6:7], AF.Sqrt)
                nc.vector.reciprocal(st[:, 7:8], st[:, 6:7])
                # (x - mean) * ln_scale, then (* rstd) + ln_bias
                nc.vector.scalar_tensor_tensor(
                    o[:], o[:], st[:, 2:3], lnsc_sb[:], ALU.subtract, ALU.mult
                )
                nc.vector.scalar_tensor_tensor(
                    o[:], o[:], st[:, 7:8], lnbi_sb[:], ALU.mult, ALU.add
                )
                nc.sync.dma_start(outp[rsl, :], o[:])

    nc.finalize()
    return nc


def _get_built():
    global _BUILT
    if _BUILT is None:
        _BUILT = _build()
    return _BUILT


def kernel(memory, decoder_input, mask, query_mask, Wk, Wv, Wq, Wf, bf, ln_scale,
           ln_bias):
    global LAST_RESULTS
    from concourse.bass_utils import run_bass_kernel_spmd

    memory = np.asarray(memory, np.float32)
    decoder_input = np.asarray(decoder_input, np.float32)
    mask = np.asarray(mask)
    query_mask = np.asarray(query_mask, np.float32)
    Wk = np.asarray(Wk, np.float32)
    Wv = np.asarray(Wv, np.float32)
    Wq = np.asarray(Wq, np.float32)
    Wf = np.asarray(Wf, np.float32)
    bf16 = ml_dtypes.bfloat16

    wqT = np.ascontiguousarray(Wq.T) * np.float32(1.0 / np.sqrt(D))
    wkT = np.ascontiguousarray(Wk.T)
    wvT = np.ascontiguousarray(Wv.T)
    wfT = np.ascontiguousarray(Wf.T).astype(bf16)
    lnsc_b = np.tile(np.asarray(ln_scale, np.float32)[None, :], (128, 1))
    lnbi_b = np.tile(np.asarray(ln_bias, np.float32)[None, :], (128, 1))

    in_maps = []
    for c in range(8):
        b, rb = c // 2, c % 2
        qsl = slice(rb * Q, (rb + 1) * Q)
        memT = np.ascontiguousarray(memory[b].T)
        decT_full = np.ascontiguousarray(decoder_input[b].T)
        decT = np.ascontiguousarray(decT_full[:, qsl])
        in_maps.append({
            "memT": memT,
            "decT": decT,
            "decTh": decT.astype(bf16),
            "dec": np.ascontiguousarray(decoder_input[b, qsl]) + np.asarray(bf, np.float32)[None, :],
            "maskq": (1.0 - mask[b, qsl].astype(np.float32)).astype(ml_dtypes.bfloat16),
            "wqT": wqT, "wkT": wkT, "wvT": wvT, "wfT": wfT,
            "qmc": np.ascontiguousarray(query_mask[b, qsl].reshape(QC, 128).T),
            "lnsc": lnsc_b, "lnbi": lnbi_b,
        })

    nc = _get_built()
    LAST_RESULTS = run_bass_kernel_spmd(nc, in_maps, core_ids=list(range(8)))
    res = LAST_RESULTS.results

    out = np.empty((B, S, H), np.float32)
    attns = np.empty((B, NH, S, S), np.float32)
    for c in range(8):
        b, rb = c // 2, c % 2
        qsl = slice(rb * Q, (rb + 1) * Q)
        out[b, qsl] = res[c]["outp"]
        # reference attns[i,j] = attn[head=i, batch=j] (torch .view regroup)
        for hh in range(NH):
            attns[hh, b, qsl, :] = res[c][f"attns{hh}"].astype(np.float32)
    return out, attns
